# revision 2
# baseline (speedup 1.0000x reference)
"""Trainium2 Bass kernel for a Chemprop GNN message-passing layer.

Reference computation (single layer, n_nodes=50000, n_edges=300000, hidden=256):
    H   = relu(E)                                  # [E, 256]
    M_v = segment_sum(H, dest, n_nodes)            # [V, 256]
    out = (M_v[src] - H[rev]) @ W.T + b            # [E, 256]

Distribution over 8 NeuronCores (zero collectives):
  * Nodes are sharded: core c owns node range [c*6250, (c+1)*6250), padded to
    49 blocks of 128 lanes.
  * Host prep does all pure data movement: relu(E), the dest-grouped
    permutation of H rows (phase 1), the rev-gather H[rev] pre-transposed
    into [d, e] layout (phase 2), and compact per-slot lane tables.
  * Phase 1 (segment sum): the device streams pre-relu'd dest-grouped H rows,
    builds one-hot selectors S[e, n] = (dest_lane[e] == n) on DVE, and
    accumulates per 128-node block via matmuls mv += S_chunk.T @ H_chunk.
    M_v lives in SBUF (49 blocks x [128, 256] f16).
  * Phase 2 (gather-subtract-linear), fully transposed dataflow:
      pvT[d, e]  = mv_blk.T-gather via matmul(lhsT=mv_blk, rhs=R) where
                   R[n, e] = (src_lane[e] == n) is built from a compact lane
                   row via gpsimd partition_broadcast + DVE is_equal.
      muvT[d, e] = pvT - H[rev].T   (erT streamed pre-transposed from host)
      outT[o, e] = sum_d W.T[d, o] * muvT[d, e]  (4 accumulating matmuls per
                   chunk pair), bias fused into the PSUM->SBUF copy on the
                   scalar engine (per-partition bias in transposed layout).
    Output is written f16 in [block][o-half][o][slot] layout; the host
    transposes back to edge-major f32.
"""

import sys
from contextlib import ExitStack

import numpy as np

sys.path.insert(0, "/opt/trn_rl_repo")

import concourse.bass as bass
import concourse.bacc as bacc
import concourse.tile as tile
from concourse import mybir
from concourse.bass_utils import run_bass_kernel_spmd

MM_DT = "f16"

N_NODES = 50000
N_EDGES = 300000
HID = 256
NC = 8
P = 128
NPC = N_NODES // NC          # 6250 nodes per core
NBLK = (NPC + P - 1) // P    # 49 blocks of 128 node lanes per core
PAD_LANE = 200.0             # sentinel lane value -> one-hot row of zeros


def _group_slots(node_ids):
    """Group edges by (core, block) of node ownership; assign (chunk, lane) slots.

    Returns (order, core, blk, j, p, lane, CPB): arrays over edges in grouped
    order; edge order[i] sits at core[i], block blk[i], chunk j[i], lane p[i],
    and selects node lane lane[i] within the block. CPB = uniform chunks/block.
    """
    c = node_ids // NPC
    loc = node_ids - c * NPC
    blk = loc >> 7
    lane = loc & 127
    g = c * NBLK + blk
    order = np.argsort(g, kind="stable")
    gs = g[order]
    starts = np.searchsorted(gs, np.arange(NC * NBLK))
    counts = np.diff(np.append(starts, node_ids.shape[0]))
    CPB = int(-(-counts.max() // P))
    rank = np.arange(node_ids.shape[0]) - starts[gs]
    j = rank >> 7
    p = rank & 127
    return order, c[order], blk[order], j, p, lane[order], int(CPB)


def prepare(E, edge_index, rev_index, W, b):
    """Host-side sharding. Returns (in_maps, meta)."""
    src = np.asarray(edge_index[0], dtype=np.int64)
    dest = np.asarray(edge_index[1], dtype=np.int64)
    rev = np.asarray(rev_index, dtype=np.int64)
    W = np.asarray(W, dtype=np.float32)
    b = np.asarray(b, dtype=np.float32)
    H = np.maximum(np.asarray(E, dtype=np.float32), 0.0).astype(np.float16)

    # ---- phase 1: dest-grouped permuted sharding of relu(E) ----
    o1, c1, blk1, j1, p1, lane1, CPB1 = _group_slots(dest)
    row1 = blk1 * (CPB1 * P) + j1 * P + p1
    col1 = blk1 * CPB1 + j1

    # ---- phase 2: src-grouped slots ----
    o2, c2, blk2, j2, p2, lane2, CPB2 = _group_slots(src)
    R1 = NBLK * CPB1 * P
    R2 = NBLK * CPB2 * P
    row2 = blk2 * (CPB2 * P) + j2 * P + p2

    Wt_stack = np.ascontiguousarray(W.T.reshape(2, P, HID)).astype(np.float16)
    bias_cols = np.ascontiguousarray(b.reshape(2, P).T)  # [128, 2] f32
    iota_rep1 = np.ascontiguousarray(
        np.tile(np.arange(P, dtype=np.float16), (P, CPB1)))
    iota_col = np.arange(P, dtype=np.float16).reshape(P, 1)

    in_maps = []
    metas = []
    for c in range(NC):
        m1 = c1 == c
        e1 = o1[m1]
        E_p1 = np.zeros((R1, HID), np.float16)
        E_p1[row1[m1]] = H[e1]
        dest_f32 = np.full((P, NBLK * CPB1), PAD_LANE, np.float16)
        dest_f32[p1[m1], col1[m1]] = lane1[m1].astype(np.float16)

        m2 = c2 == c
        e2 = o2[m2]
        # H[rev] pre-transposed: Ert[(bb, t, d), (j, p)] = H[rev[e], t*128+d]
        G = np.zeros((NBLK, 2, P, CPB2, P), np.float16)
        G[blk2[m2], :, :, j2[m2], p2[m2]] = (
            H[rev[e2]].reshape(-1, 2, P))
        Ert = np.ascontiguousarray(G.reshape(NBLK * 2 * P, CPB2 * P))
        src_row = np.full((1, R2), PAD_LANE, np.float16)
        src_row[0, row2[m2]] = lane2[m2].astype(np.float16)

        in_maps.append({
            "E_p1": E_p1,
            "dest_f32": dest_f32,
            "Ert": Ert,
            "src_row": src_row,
            "Wt": Wt_stack,
            "bias_cols": bias_cols,
            "iota_rep1": iota_rep1,
            "iota_col": iota_col,
        })
        metas.append({"e2": e2, "row2": row2[m2]})

    meta = {"CPB1": CPB1, "CPB2": CPB2, "metas": metas}
    return in_maps, meta


def build_program(CPB1, CPB2, reps=1):
    R1 = NBLK * CPB1 * P
    R2 = NBLK * CPB2 * P
    f32 = mybir.dt.float32
    f16 = mybir.dt.float16
    nc = bacc.Bacc("TRN2", target_bir_lowering=False, debug=False,
                   num_devices=NC)
    E_p1 = nc.dram_tensor("E_p1", [R1, HID], f16, kind="ExternalInput").ap()
    dest_f32 = nc.dram_tensor("dest_f32", [P, NBLK * CPB1], f16,
                              kind="ExternalInput").ap()
    Ert = nc.dram_tensor("Ert", [NBLK * 2 * P, CPB2 * P], f16,
                         kind="ExternalInput").ap()
    src_row = nc.dram_tensor("src_row", [1, R2], f16,
                             kind="ExternalInput").ap()
    Wt = nc.dram_tensor("Wt", [2, P, HID], f16, kind="ExternalInput").ap()
    bias_cols = nc.dram_tensor("bias_cols", [P, 2], f32,
                               kind="ExternalInput").ap()
    iota_rep1 = nc.dram_tensor("iota_rep1", [P, CPB1 * P], f16,
                               kind="ExternalInput").ap()
    iota_col = nc.dram_tensor("iota_col", [P, 1], f16,
                              kind="ExternalInput").ap()
    out = nc.dram_tensor("out", [NBLK * 2 * P, CPB2 * P], f16,
                         kind="ExternalOutput").ap()

    with tile.TileContext(nc) as tc:
        with ExitStack() as ctx:
            const = ctx.enter_context(tc.tile_pool(name="const", bufs=1))
            sb = ctx.enter_context(tc.tile_pool(name="sb", bufs=4))
            mvp = ctx.enter_context(tc.tile_pool(name="mv", bufs=1))
            ps_mv = ctx.enter_context(
                tc.tile_pool(name="ps_mv", bufs=2, space="PSUM"))
            ps_pv = ctx.enter_context(
                tc.tile_pool(name="ps_pv", bufs=2, space="PSUM"))
            ps_out = ctx.enter_context(
                tc.tile_pool(name="ps_out", bufs=2, space="PSUM"))

            # constants
            wt0 = const.tile([P, HID], f16)
            nc.sync.dma_start(out=wt0[:], in_=Wt[0])
            wt1 = const.tile([P, HID], f16)
            nc.sync.dma_start(out=wt1[:], in_=Wt[1])
            bias_t = const.tile([P, 2], f32)
            nc.sync.dma_start(out=bias_t[:], in_=bias_cols[:])
            iota_r1 = const.tile([P, CPB1 * P], f16)
            nc.sync.dma_start(out=iota_r1[:], in_=iota_rep1[:])
            iota_c = const.tile([P, 1], f16)
            nc.sync.dma_start(out=iota_c[:], in_=iota_col[:])
            dest_t = const.tile([P, NBLK * CPB1], f16)
            nc.sync.dma_start(out=dest_t[:], in_=dest_f32[:])
            src_t = const.tile([1, R2], f16)
            nc.sync.dma_start(out=src_t[:], in_=src_row[:])

            mv_all = mvp.tile([P, NBLK * HID], f16)  # resident M_v

            for _rep in range(reps):
                _emit_body(nc, tc, locals(), CPB1, CPB2)
    nc.compile()
    return nc


def _emit_body(nc, tc, env, CPB1, CPB2):
    f32 = mybir.dt.float32
    f16 = mybir.dt.float16
    sb, mv_all = env["sb"], env["mv_all"]
    ps_mv, ps_pv, ps_out = env["ps_mv"], env["ps_pv"], env["ps_out"]
    E_p1, Ert, out = env["E_p1"], env["Ert"], env["out"]
    dest_t, src_t = env["dest_t"], env["src_t"]
    iota_r1, iota_c = env["iota_r1"], env["iota_c"]
    wt0, wt1, bias_t = env["wt0"], env["wt1"], env["bias_t"]
    EW2 = CPB2 * P  # phase-2 slot columns per block
    for bb in range(NBLK):
        # ---------------- phase 1: segment sum ----------------
        h_blk = sb.tile([P, CPB1 * HID], f16, tag="h_blk")
        base1 = bb * CPB1 * P
        nc.sync.dma_start(
            out=h_blk[:].rearrange("p (j d) -> p j d", j=CPB1),
            in_=E_p1[base1:base1 + CPB1 * P, :].rearrange(
                "(j p) d -> p j d", p=P))
        s_all = sb.tile([P, CPB1 * P], f16, tag="s_all")
        nc.vector.tensor_tensor(
            out=s_all[:].rearrange("p (j n) -> p j n", j=CPB1),
            in0=dest_t[:, bb * CPB1:(bb + 1) * CPB1].to_broadcast(
                [P, CPB1, P]),
            in1=iota_r1[:].rearrange("p (j n) -> p j n", j=CPB1),
            op=mybir.AluOpType.is_equal)
        mv_ps = ps_mv.tile([P, HID], f32, space="PSUM")
        for j in range(CPB1):
            nc.tensor.matmul(
                out=mv_ps[:], lhsT=s_all[:, j * P:(j + 1) * P],
                rhs=h_blk[:, j * HID:(j + 1) * HID],
                start=(j == 0), stop=(j == CPB1 - 1))
        nc.scalar.copy(out=mv_all[:, bb * HID:(bb + 1) * HID], in_=mv_ps[:])

        # ------ phase 2: transposed gather-subtract-linear (same block) ------
        ert_blk = sb.tile([P, 2 * EW2], f16, tag="ert_blk")
        nc.sync.dma_start(
            out=ert_blk[:].rearrange("p (t e) -> p t e", t=2),
            in_=Ert[bb * HID:(bb + 1) * HID, :].rearrange(
                "(t p) e -> p t e", p=P))
        sbc = sb.tile([P, EW2], f16, tag="sbc")
        nc.gpsimd.partition_broadcast(
            sbc[:], src_t[0:1, bb * EW2:(bb + 1) * EW2])
        r_all = sb.tile([P, EW2], f16, tag="r_all")
        nc.vector.tensor_tensor(
            out=r_all[:], in0=sbc[:],
            in1=iota_c[:, 0:1].to_broadcast([P, EW2]),
            op=mybir.AluOpType.is_equal)
        out_blk = sb.tile([P, 2 * EW2], f16, tag="out_blk")
        mv_lo = mv_all[:, bb * HID:bb * HID + P]
        mv_hi = mv_all[:, bb * HID + P:(bb + 1) * HID]
        for j0 in range(0, CPB2, 2):
            w = min(2 * P, EW2 - j0 * P)  # 256, or 128 for odd tail
            ec = j0 * P
            pv_ps = ps_pv.tile([P, 2 * w], f32, space="PSUM")
            nc.tensor.matmul(out=pv_ps[:, 0:w], lhsT=mv_lo,
                             rhs=r_all[:, ec:ec + w], start=True, stop=True)
            nc.tensor.matmul(out=pv_ps[:, w:2 * w], lhsT=mv_hi,
                             rhs=r_all[:, ec:ec + w], start=True, stop=True)
            muv = sb.tile([P, 2 * w], f16, tag="muv")
            nc.vector.tensor_tensor(
                out=muv[:, 0:w], in0=pv_ps[:, 0:w],
                in1=ert_blk[:, ec:ec + w], op=mybir.AluOpType.subtract)
            nc.vector.tensor_tensor(
                out=muv[:, w:2 * w], in0=pv_ps[:, w:2 * w],
                in1=ert_blk[:, EW2 + ec:EW2 + ec + w],
                op=mybir.AluOpType.subtract)
            o_ps = ps_out.tile([P, 2 * w], f32, space="PSUM")
            nc.tensor.matmul(out=o_ps[:, 0:w], lhsT=wt0[:, 0:P],
                             rhs=muv[:, 0:w], start=True, stop=False)
            nc.tensor.matmul(out=o_ps[:, 0:w], lhsT=wt1[:, 0:P],
                             rhs=muv[:, w:2 * w], start=False, stop=True)
            nc.tensor.matmul(out=o_ps[:, w:2 * w], lhsT=wt0[:, P:HID],
                             rhs=muv[:, 0:w], start=True, stop=False)
            nc.tensor.matmul(out=o_ps[:, w:2 * w], lhsT=wt1[:, P:HID],
                             rhs=muv[:, w:2 * w], start=False, stop=True)
            nc.scalar.activation(
                out_blk[:, ec:ec + w], o_ps[:, 0:w],
                mybir.ActivationFunctionType.Identity,
                bias=bias_t[:, 0:1])
            nc.scalar.activation(
                out_blk[:, EW2 + ec:EW2 + ec + w], o_ps[:, w:2 * w],
                mybir.ActivationFunctionType.Identity,
                bias=bias_t[:, 1:2])
        nc.sync.dma_start(
            out=out[bb * HID:(bb + 1) * HID, :].rearrange(
                "(t p) e -> p t e", p=P),
            in_=out_blk[:].rearrange("p (t e) -> p t e", t=2))


def assemble(results, meta):
    CPB2 = meta["CPB2"]
    out_full = np.empty((N_EDGES, HID), np.float32)
    for c in range(NC):
        mc = meta["metas"][c]
        arr = np.asarray(results[c]["out"]).reshape(NBLK, 2, P, CPB2 * P)
        # -> slot-major [ (bb, slot), (t, o) ]
        arr = arr.transpose(0, 3, 1, 2).reshape(NBLK * CPB2 * P, HID)
        out_full[mc["e2"]] = arr[mc["row2"]].astype(np.float32)
    return out_full


def kernel(E, edge_index, rev_index, W, b):
    in_maps, meta = prepare(E, edge_index, rev_index, W, b)
    nc = build_program(meta["CPB1"], meta["CPB2"])
    res = run_bass_kernel_spmd(nc, in_maps, list(range(NC)))
    return assemble(res.results, meta)


# revision 3
# speedup vs baseline: 2.4444x; 2.4444x over previous
"""Trainium2 Bass kernel for a Chemprop GNN message-passing layer.

Reference computation (single layer, n_nodes=50000, n_edges=300000, hidden=256):
    H   = relu(E)                                  # [E, 256]
    M_v = segment_sum(H, dest, n_nodes)            # [V, 256]
    out = (M_v[src] - H[rev]) @ W.T + b            # [E, 256]

Distribution over 8 NeuronCores (zero collectives):
  * Nodes are sharded: core c owns node range [c*6250, (c+1)*6250), padded to
    49 blocks of 128 lanes.
  * Host prep does all pure data movement: relu(E), the dest-grouped
    permutation of H rows (phase 1), the rev-gather H[rev] pre-transposed
    into [d, e] layout (phase 2), and compact per-slot lane tables.
  * Phase 1 (segment sum): the device streams pre-relu'd dest-grouped H rows,
    builds one-hot selectors S[e, n] = (dest_lane[e] == n) on DVE, and
    accumulates per 128-node block via matmuls mv += S_chunk.T @ H_chunk.
    M_v lives in SBUF (49 blocks x [128, 256] f16).
  * Phase 2 (gather-subtract-linear), fully transposed dataflow:
      pvT[d, e]  = mv_blk.T-gather via matmul(lhsT=mv_blk, rhs=R) where
                   R[n, e] = (src_lane[e] == n) is built from a compact lane
                   row via gpsimd partition_broadcast + DVE is_equal.
      muvT[d, e] = pvT - H[rev].T   (erT streamed pre-transposed from host)
      outT[o, e] = sum_d W.T[d, o] * muvT[d, e]  (4 accumulating matmuls per
                   chunk pair), bias fused into the PSUM->SBUF copy on the
                   scalar engine (per-partition bias in transposed layout).
    Output is written f16 in [block][o-half][o][slot] layout; the host
    transposes back to edge-major f32.
"""

import sys
from contextlib import ExitStack

import numpy as np

sys.path.insert(0, "/opt/trn_rl_repo")

import concourse.bass as bass
import concourse.bacc as bacc
import concourse.tile as tile
from concourse import mybir
from concourse.bass_utils import run_bass_kernel_spmd

MM_DT = "f16"

# timing-only ablation switches (break correctness when nonzero)
ABL_NO_H = False     # skip phase-1 h DMA + segment-sum matmuls
ABL_NO_ERT = False   # skip ert DMA + subtract
ABL_NO_R = False     # skip partition_broadcast + is_equal (r from const)
ABL_NO_LIN = False   # skip outT matmuls + ACT bias copies
ABL_NO_OUT = False   # skip output DMA
PACK = True          # degree-aware node->block packing (CPB 7 -> 6)

N_NODES = 50000
N_EDGES = 300000
HID = 256
NC = 8
P = 128
NPC = N_NODES // NC          # 6250 nodes per core
NBLK = (NPC + P - 1) // P    # 49 blocks of 128 node lanes per core
PAD_LANE = 200.0             # sentinel lane value -> one-hot row of zeros


def _pack_nodes(d1, d2):
    """Assign nodes to (core, blk, lane) so each 128-node block has
    dest-degree sum and src-degree sum both <= cap, minimizing the uniform
    chunks-per-block. Returns (core_of, blk_of, lane_of) arrays [N_NODES]."""
    nbins = NC * NBLK
    order = np.argsort(-(d1 + d2), kind="stable")
    cnt = np.zeros(nbins, np.int32)
    s1 = np.zeros(nbins, np.int64)
    s2 = np.zeros(nbins, np.int64)
    binof = np.empty(N_NODES, np.int32)
    cap = 6 * P
    for v in order:
        a, b_ = int(d1[v]), int(d2[v])
        load = np.maximum(s1 + a, s2 + b_)
        load[cnt >= P] = 1 << 40
        # prefer bins that stay under cap on both axes
        feas = (s1 + a <= cap) & (s2 + b_ <= cap) & (cnt < P)
        if feas.any():
            cand = np.where(feas)[0]
            k = cand[np.argmax(np.maximum(s1[cand] + a, s2[cand] + b_))]
        else:
            k = int(np.argmin(load))
        binof[v] = k
        cnt[k] += 1
        s1[k] += a
        s2[k] += b_
    core_of = binof // NBLK
    blk_of = binof % NBLK
    lane_of = np.zeros(N_NODES, np.int32)
    # assign lanes within each bin by arrival order
    seen = np.zeros(nbins, np.int32)
    for v in order:
        k = binof[v]
        lane_of[v] = seen[k]
        seen[k] += 1
    return core_of.astype(np.int64), blk_of.astype(np.int64), \
        lane_of.astype(np.int64)


def _group_slots(node_ids, node_map=None):
    """Group edges by (core, block) of node ownership; assign (chunk, lane) slots.

    Returns (order, core, blk, j, p, lane, CPB): arrays over edges in grouped
    order; edge order[i] sits at core[i], block blk[i], chunk j[i], lane p[i],
    and selects node lane lane[i] within the block. CPB = uniform chunks/block.
    """
    if node_map is None:
        c = node_ids // NPC
        loc = node_ids - c * NPC
        blk = loc >> 7
        lane = loc & 127
    else:
        core_of, blk_of, lane_of = node_map
        c = core_of[node_ids]
        blk = blk_of[node_ids]
        lane = lane_of[node_ids]
    g = c * NBLK + blk
    order = np.argsort(g, kind="stable")
    gs = g[order]
    starts = np.searchsorted(gs, np.arange(NC * NBLK))
    counts = np.diff(np.append(starts, node_ids.shape[0]))
    CPB = int(-(-counts.max() // P))
    rank = np.arange(node_ids.shape[0]) - starts[gs]
    j = rank >> 7
    p = rank & 127
    return order, c[order], blk[order], j, p, lane[order], int(CPB)


def prepare(E, edge_index, rev_index, W, b):
    """Host-side sharding. Returns (in_maps, meta)."""
    src = np.asarray(edge_index[0], dtype=np.int64)
    dest = np.asarray(edge_index[1], dtype=np.int64)
    rev = np.asarray(rev_index, dtype=np.int64)
    W = np.asarray(W, dtype=np.float32)
    b = np.asarray(b, dtype=np.float32)
    H = np.maximum(np.asarray(E, dtype=np.float32), 0.0).astype(np.float16)

    # ---- phase 1: dest-grouped permuted sharding of relu(E) ----
    o1, c1, blk1, j1, p1, lane1, CPB1 = _group_slots(dest)
    row1 = blk1 * (CPB1 * P) + j1 * P + p1
    col1 = blk1 * CPB1 + j1

    # ---- phase 2: src-grouped slots ----
    o2, c2, blk2, j2, p2, lane2, CPB2 = _group_slots(src)
    R1 = NBLK * CPB1 * P
    R2 = NBLK * CPB2 * P
    row2 = blk2 * (CPB2 * P) + j2 * P + p2

    Wt_stack = np.ascontiguousarray(W.T.reshape(2, P, HID)).astype(np.float16)
    bias_cols = np.ascontiguousarray(b.reshape(2, P).T)  # [128, 2] f32
    iota_rep1 = np.ascontiguousarray(
        np.tile(np.arange(P, dtype=np.float16), (P, CPB1)))
    iota_col = np.arange(P, dtype=np.float16).reshape(P, 1)

    in_maps = []
    metas = []
    for c in range(NC):
        m1 = c1 == c
        e1 = o1[m1]
        E_p1 = np.zeros((R1, HID), np.float16)
        E_p1[row1[m1]] = H[e1]
        dest_f32 = np.full((P, NBLK * CPB1), PAD_LANE, np.float16)
        dest_f32[p1[m1], col1[m1]] = lane1[m1].astype(np.float16)

        m2 = c2 == c
        e2 = o2[m2]
        # H[rev] pre-transposed: Ert[(bb, t, d), (j, p)] = H[rev[e], t*128+d]
        G = np.zeros((NBLK, 2, P, CPB2, P), np.float16)
        G[blk2[m2], :, :, j2[m2], p2[m2]] = (
            H[rev[e2]].reshape(-1, 2, P))
        Ert = np.ascontiguousarray(G.reshape(NBLK * 2 * P, CPB2 * P))
        src_row = np.full((1, R2), PAD_LANE, np.float16)
        src_row[0, row2[m2]] = lane2[m2].astype(np.float16)

        in_maps.append({
            "E_p1": E_p1,
            "dest_f32": dest_f32,
            "Ert": Ert,
            "src_row": src_row,
            "Wt": Wt_stack,
            "bias_cols": bias_cols,
            "iota_rep1": iota_rep1,
            "iota_col": iota_col,
        })
        metas.append({"e2": e2, "row2": row2[m2]})

    meta = {"CPB1": CPB1, "CPB2": CPB2, "metas": metas}
    return in_maps, meta


def build_program(CPB1, CPB2, reps=1):
    R1 = NBLK * CPB1 * P
    R2 = NBLK * CPB2 * P
    f32 = mybir.dt.float32
    f16 = mybir.dt.float16
    nc = bacc.Bacc("TRN2", target_bir_lowering=False, debug=False,
                   num_devices=NC)
    E_p1 = nc.dram_tensor("E_p1", [R1, HID], f16, kind="ExternalInput").ap()
    dest_f32 = nc.dram_tensor("dest_f32", [P, NBLK * CPB1], f16,
                              kind="ExternalInput").ap()
    Ert = nc.dram_tensor("Ert", [NBLK * 2 * P, CPB2 * P], f16,
                         kind="ExternalInput").ap()
    src_row = nc.dram_tensor("src_row", [1, R2], f16,
                             kind="ExternalInput").ap()
    Wt = nc.dram_tensor("Wt", [2, P, HID], f16, kind="ExternalInput").ap()
    bias_cols = nc.dram_tensor("bias_cols", [P, 2], f32,
                               kind="ExternalInput").ap()
    iota_rep1 = nc.dram_tensor("iota_rep1", [P, CPB1 * P], f16,
                               kind="ExternalInput").ap()
    iota_col = nc.dram_tensor("iota_col", [P, 1], f16,
                              kind="ExternalInput").ap()
    out = nc.dram_tensor("out", [NBLK * 2 * P, CPB2 * P], f16,
                         kind="ExternalOutput").ap()

    with tile.TileContext(nc) as tc:
        with ExitStack() as ctx:
            const = ctx.enter_context(tc.tile_pool(name="const", bufs=1))
            sb = ctx.enter_context(tc.tile_pool(name="sb", bufs=4))
            mvp = ctx.enter_context(tc.tile_pool(name="mv", bufs=1))
            ps_mv = ctx.enter_context(
                tc.tile_pool(name="ps_mv", bufs=2, space="PSUM"))
            ps_pv = ctx.enter_context(
                tc.tile_pool(name="ps_pv", bufs=2, space="PSUM"))
            ps_out = ctx.enter_context(
                tc.tile_pool(name="ps_out", bufs=2, space="PSUM"))

            # constants
            wt0 = const.tile([P, HID], f16)
            nc.sync.dma_start(out=wt0[:], in_=Wt[0])
            wt1 = const.tile([P, HID], f16)
            nc.sync.dma_start(out=wt1[:], in_=Wt[1])
            bias_t = const.tile([P, 2], f32)
            nc.sync.dma_start(out=bias_t[:], in_=bias_cols[:])
            iota_r1 = const.tile([P, CPB1 * P], f16)
            nc.sync.dma_start(out=iota_r1[:], in_=iota_rep1[:])
            iota_c = const.tile([P, 1], f16)
            nc.sync.dma_start(out=iota_c[:], in_=iota_col[:])
            dest_t = const.tile([P, NBLK * CPB1], f16)
            nc.sync.dma_start(out=dest_t[:], in_=dest_f32[:])
            src_t = const.tile([1, R2], f16)
            nc.sync.dma_start(out=src_t[:], in_=src_row[:])

            mv_all = mvp.tile([P, NBLK * HID], f16)  # resident M_v

            for _rep in range(reps):
                _emit_body(nc, tc, locals(), CPB1, CPB2)
    nc.compile()
    return nc


def _emit_body(nc, tc, env, CPB1, CPB2):
    f32 = mybir.dt.float32
    f16 = mybir.dt.float16
    sb, mv_all = env["sb"], env["mv_all"]
    ps_mv, ps_pv, ps_out = env["ps_mv"], env["ps_pv"], env["ps_out"]
    E_p1, Ert, out = env["E_p1"], env["Ert"], env["out"]
    dest_t, src_t = env["dest_t"], env["src_t"]
    iota_r1, iota_c = env["iota_r1"], env["iota_c"]
    wt0, wt1, bias_t = env["wt0"], env["wt1"], env["bias_t"]
    EW2 = CPB2 * P  # phase-2 slot columns per block
    for bb in range(NBLK):
        # ---------------- phase 1: segment sum ----------------
        h_blk = sb.tile([P, CPB1 * HID], f16, tag="h_blk")
        base1 = bb * CPB1 * P
        nc.sync.dma_start(
            out=h_blk[:].rearrange("p (j d) -> p j d", j=CPB1),
            in_=E_p1[base1:base1 + CPB1 * P, :].rearrange(
                "(j p) d -> p j d", p=P))
        s_all = sb.tile([P, CPB1 * P], f16, tag="s_all")
        nc.vector.tensor_tensor(
            out=s_all[:].rearrange("p (j n) -> p j n", j=CPB1),
            in0=dest_t[:, bb * CPB1:(bb + 1) * CPB1].to_broadcast(
                [P, CPB1, P]),
            in1=iota_r1[:].rearrange("p (j n) -> p j n", j=CPB1),
            op=mybir.AluOpType.is_equal)
        mv_ps = ps_mv.tile([P, HID], f32, space="PSUM")
        for j in range(CPB1):
            nc.tensor.matmul(
                out=mv_ps[:], lhsT=s_all[:, j * P:(j + 1) * P],
                rhs=h_blk[:, j * HID:(j + 1) * HID],
                start=(j == 0), stop=(j == CPB1 - 1))
        nc.scalar.copy(out=mv_all[:, bb * HID:(bb + 1) * HID], in_=mv_ps[:])

        # ------ phase 2: transposed gather-subtract-linear (same block) ------
        ert_blk = sb.tile([P, 2 * EW2], f16, tag="ert_blk")
        nc.sync.dma_start(
            out=ert_blk[:].rearrange("p (t e) -> p t e", t=2),
            in_=Ert[bb * HID:(bb + 1) * HID, :].rearrange(
                "(t p) e -> p t e", p=P))
        sbc = sb.tile([P, EW2], f16, tag="sbc")
        nc.gpsimd.partition_broadcast(
            sbc[:], src_t[0:1, bb * EW2:(bb + 1) * EW2])
        r_all = sb.tile([P, EW2], f16, tag="r_all")
        nc.vector.tensor_tensor(
            out=r_all[:], in0=sbc[:],
            in1=iota_c[:, 0:1].to_broadcast([P, EW2]),
            op=mybir.AluOpType.is_equal)
        out_blk = sb.tile([P, 2 * EW2], f16, tag="out_blk")
        mv_lo = mv_all[:, bb * HID:bb * HID + P]
        mv_hi = mv_all[:, bb * HID + P:(bb + 1) * HID]
        for j0 in range(0, CPB2, 2):
            w = min(2 * P, EW2 - j0 * P)  # 256, or 128 for odd tail
            ec = j0 * P
            pv_ps = ps_pv.tile([P, 2 * w], f32, space="PSUM")
            nc.tensor.matmul(out=pv_ps[:, 0:w], lhsT=mv_lo,
                             rhs=r_all[:, ec:ec + w], start=True, stop=True)
            nc.tensor.matmul(out=pv_ps[:, w:2 * w], lhsT=mv_hi,
                             rhs=r_all[:, ec:ec + w], start=True, stop=True)
            muv = sb.tile([P, 2 * w], f16, tag="muv")
            nc.vector.tensor_tensor(
                out=muv[:, 0:w], in0=pv_ps[:, 0:w],
                in1=ert_blk[:, ec:ec + w], op=mybir.AluOpType.subtract)
            nc.vector.tensor_tensor(
                out=muv[:, w:2 * w], in0=pv_ps[:, w:2 * w],
                in1=ert_blk[:, EW2 + ec:EW2 + ec + w],
                op=mybir.AluOpType.subtract)
            o_ps = ps_out.tile([P, 2 * w], f32, space="PSUM")
            nc.tensor.matmul(out=o_ps[:, 0:w], lhsT=wt0[:, 0:P],
                             rhs=muv[:, 0:w], start=True, stop=False)
            nc.tensor.matmul(out=o_ps[:, 0:w], lhsT=wt1[:, 0:P],
                             rhs=muv[:, w:2 * w], start=False, stop=True)
            nc.tensor.matmul(out=o_ps[:, w:2 * w], lhsT=wt0[:, P:HID],
                             rhs=muv[:, 0:w], start=True, stop=False)
            nc.tensor.matmul(out=o_ps[:, w:2 * w], lhsT=wt1[:, P:HID],
                             rhs=muv[:, w:2 * w], start=False, stop=True)
            nc.scalar.activation(
                out_blk[:, ec:ec + w], o_ps[:, 0:w],
                mybir.ActivationFunctionType.Identity,
                bias=bias_t[:, 0:1])
            nc.scalar.activation(
                out_blk[:, EW2 + ec:EW2 + ec + w], o_ps[:, w:2 * w],
                mybir.ActivationFunctionType.Identity,
                bias=bias_t[:, 1:2])
        nc.sync.dma_start(
            out=out[bb * HID:(bb + 1) * HID, :].rearrange(
                "(t p) e -> p t e", p=P),
            in_=out_blk[:].rearrange("p (t e) -> p t e", t=2))


def assemble(results, meta):
    CPB2 = meta["CPB2"]
    out_full = np.empty((N_EDGES, HID), np.float32)
    for c in range(NC):
        mc = meta["metas"][c]
        arr = np.asarray(results[c]["out"]).reshape(NBLK, 2, P, CPB2 * P)
        # -> slot-major [ (bb, slot), (t, o) ]
        arr = arr.transpose(0, 3, 1, 2).reshape(NBLK * CPB2 * P, HID)
        out_full[mc["e2"]] = arr[mc["row2"]].astype(np.float32)
    return out_full


def kernel(E, edge_index, rev_index, W, b):
    in_maps, meta = prepare(E, edge_index, rev_index, W, b)
    nc = build_program(meta["CPB1"], meta["CPB2"])
    res = run_bass_kernel_spmd(nc, in_maps, list(range(NC)))
    return assemble(res.results, meta)
